# revision 8
# baseline (speedup 1.0000x reference)
"""Multi-head attention (B=2, S=4096, DM=512, H=8) on 8 trn2 NeuronCores.

Sharding: data + head parallel. Core c handles batch b = c//4 and head pair
hp = c%4 (heads 2hp, 2hp+1 = a 128-wide slice of the model dim). Each core
computes its two heads' full attention plus the partial output projection
(its 128 rows of Wo); the host sums the 4 partials per batch and adds bo.

v5:
  * Key compaction: the padding mask is known on the host, and masked keys
    get softmax weight exactly 0 in the reference (logit - 1e9). k/v are
    compacted to the unmasked keys (padded to whole 128-key blocks, zero
    rows beyond): NSKP ~ 29 blocks instead of 32.
  * One [128, 1024] logits tile per j (both heads side by side) from a
    3-deep PSUM pool that also serves the Wo partials and the projection
    scratch (every allocation is the same 2-bank shape, so the Wo/proj
    spikes do not steal logits depth unpredictably). Depth 3 hides the
    ~450ns PE-drain+semaphore latency of the QK->exp handoff that bound
    v2 (depth 2) at ~1130ns/j.
  * Whole-tile exp alternating per j: even j on ScalarE (exact spline Exp),
    odd j on the DVE (one-op Schraudolph int16-bitcast fast-exp). The two
    engines work on consecutive j concurrently; one instruction per tile
    keeps the per-instruction constants (352cyc ScalarE / 120cyc DVE) at
    half the per-head-split cost.
  * QK pair and Wo pair issued adjacently with explicit 64x128 tile
    positions (0,0)/(64,0).
  * ScalarE absorbs the acc->stg copies, the K-proj copy, and the
    per-partition rowsum scale of pso0; the DVE keeps the fast-exp, the
    stt combine, the Q-proj copy and the bf16 casts. V-proj mask scaling
    alternates engines (chunk-0 relief).
"""
import numpy as np
import ml_dtypes

import concourse.bass as bass
from concourse import bacc
import concourse.mybir as mybir
import concourse.tile as tile
from concourse import bass_utils
from concourse.alu_op_type import AluOpType

FP32 = mybir.dt.float32
BF16 = mybir.dt.bfloat16
I16 = mybir.dt.int16
AF = mybir.ActivationFunctionType

B, S, DM, H = 2, 4096, 512, 8
D = DM // H              # 64
NCORES = 8
CHUNK = 512              # q columns processed per attention chunk
NCH = S // CHUNK         # 8
NT = DM // 128           # 4 dm tiles

_CACHE = {}


def _build(with_bias, nskp):
    """nskp = number of 128-key blocks after compaction (<= 32)."""
    kch = (nskp + 3) // 4         # 512-key chunks for K/V (last may be partial)
    nc = bacc.Bacc("TRN2", target_bir_lowering=False, debug=False)

    # chunk-major: [ch, p, t, c] so each chunk's DMA has 4KB-contiguous rows
    qT = nc.dram_tensor("qT", [NCH, 128, NT, CHUNK], BF16, kind="ExternalInput")
    kT = nc.dram_tensor("kT", [kch, 128, NT, CHUNK], BF16, kind="ExternalInput")
    vT = nc.dram_tensor("vT", [kch, 128, NT, CHUNK], BF16, kind="ExternalInput")
    m01 = nc.dram_tensor("m01", [128, nskp], FP32, kind="ExternalInput")
    wq = nc.dram_tensor("wq", [DM, 128], BF16, kind="ExternalInput")
    wk = nc.dram_tensor("wk", [DM, 128], BF16, kind="ExternalInput")
    wv = nc.dram_tensor("wv", [DM, 130], BF16, kind="ExternalInput")
    bqk = nc.dram_tensor("bqk", [1, 256], BF16, kind="ExternalInput")  # bq|bk
    bv = nc.dram_tensor("bv", [1, 130], BF16, kind="ExternalInput")
    wo = nc.dram_tensor("wo", [128, DM], BF16, kind="ExternalInput")
    out = nc.dram_tensor("out", [S, DM], FP32, kind="ExternalOutput")

    def kblks(g):  # blocks in K/V group g
        return min(4, nskp - 4 * g)

    # wo_combine slots inside the next chunk's j loop: 4 evenly spaced js
    gap = max(1, (nskp - 5) // 3)
    wo_slot = {4 + i * gap: i for i in range(4)}
    assert len(wo_slot) == 4 and max(wo_slot) <= nskp - 1

    with tile.TileContext(nc) as tc:
        with tc.tile_pool(name="consts", bufs=1) as consts, \
             tc.tile_pool(name="acts", bufs=1) as acts:
            # ---- first chunk's activations land before anything else ----
            qT_sb = acts.tile([128, NCH, NT, CHUNK], BF16)
            kT_sb = acts.tile([128, kch, NT, CHUNK], BF16)
            vT_sb = acts.tile([128, kch, NT, CHUNK], BF16)
            for p0 in (0, 64):
                nc.sync.dma_start(out=kT_sb[p0:p0 + 64, 0], in_=kT[0][p0:p0 + 64])
                nc.sync.dma_start(out=vT_sb[p0:p0 + 64, 0], in_=vT[0][p0:p0 + 64])
                nc.sync.dma_start(out=qT_sb[p0:p0 + 64, 0], in_=qT[0][p0:p0 + 64])

            # ---- tiny constants; warm the Exp table set during the DMA phase ----
            ones_sb = consts.tile([1, CHUNK], BF16)
            nc.vector.memset(ones_sb, 1.0)
            warm = consts.tile([1, 1], FP32)
            nc.scalar.activation(warm, ones_sb[0:1, 0:1], AF.Exp)

            wq_sb = consts.tile([128, NT, 128], BF16)
            wk_sb = consts.tile([128, NT, 128], BF16)
            wv_sb = consts.tile([128, NT, 130], BF16)
            bqk_sb = consts.tile([1, 256], BF16)
            bv_sb = consts.tile([1, 130], BF16)
            wo_sb = consts.tile([128, DM], BF16)     # rows 0:64 h0, 64:128 h1
            m01_sb = consts.tile([128, nskp], FP32)
            for t in range(NT):
                nc.sync.dma_start(out=wk_sb[:, t, :], in_=wk[t * 128:(t + 1) * 128, :])
                nc.sync.dma_start(out=wv_sb[:, t, :], in_=wv[t * 128:(t + 1) * 128, :])
                nc.sync.dma_start(out=wq_sb[:, t, :], in_=wq[t * 128:(t + 1) * 128, :])
            nc.sync.dma_start(out=bv_sb, in_=bv[:, :])
            nc.sync.dma_start(out=m01_sb, in_=m01[:, :])
            if with_bias:
                nc.sync.dma_start(out=bqk_sb, in_=bqk[:, :])

            # ---- remaining activation DMAs ----
            nc.sync.dma_start(out=wo_sb, in_=wo[:, :])
            for ch in range(1, kch):
                cols = CHUNK if ch < kch - 1 else (nskp - 4 * (kch - 1)) * 128
                nc.sync.dma_start(out=kT_sb[:, ch, :, 0:cols], in_=kT[ch][:, :, 0:cols])
                nc.sync.dma_start(out=vT_sb[:, ch, :, 0:cols], in_=vT[ch][:, :, 0:cols])
            for ch in range(1, NCH):
                nc.sync.dma_start(out=qT_sb[:, ch], in_=qT[ch])

            QhT = acts.tile([128, S], BF16)
            KhT = acts.tile([128, kch * CHUNK], BF16)
            VA = acts.tile([128, nskp, 130], BF16)

            with tc.tile_pool(name="plog", bufs=2, space="PSUM") as pl, \
                 tc.tile_pool(name="pwo", bufs=1, space="PSUM") as pw, \
                 tc.tile_pool(name="pacc", bufs=1, space="PSUM") as pacc, \
                 tc.tile_pool(name="sexp", bufs=4) as sexp, \
                 tc.tile_pool(name="sout", bufs=3) as sout, \
                 tc.tile_pool(name="sow", bufs=2) as sow, \
                 tc.tile_pool(name="srs", bufs=2) as srs:

                def proj_qk(dst, w_sb, brow, x_sb, ch, cols=CHUNK, on_dve=True):
                    psb = pw.tile([128, 2 * CHUNK], FP32, tag="ps")
                    ps = psb[:, 0:CHUNK]
                    sl = bass.ds(ch * CHUNK, cols)
                    for t in range(NT):
                        nc.tensor.matmul(ps[:, 0:cols], w_sb[:, t, :],
                                         x_sb[:, ch, t, 0:cols],
                                         start=(t == 0),
                                         stop=(t == NT - 1 and not with_bias))
                    if with_bias:
                        nc.tensor.matmul(ps[:, 0:cols], brow, ones_sb[:, 0:cols],
                                         start=False, stop=True)
                    if on_dve:
                        nc.vector.tensor_copy(dst[:, sl], ps[:, 0:cols])
                    else:
                        nc.scalar.copy(dst[:, sl], ps[:, 0:cols])

                def proj_k(g):
                    proj_qk(KhT, wk_sb, bqk_sb[0:1, 128:256], kT_sb, g,
                            kblks(g) * 128, on_dve=False)

                def proj_v(j, psv):
                    # V bias matmul always runs: it also writes the ones columns
                    # (cols 64/129) that produce the attention rowsums.
                    for t in range(NT):
                        nc.tensor.matmul(psv[:, 0:130],
                                         vT_sb[:, j // 4, t,
                                               (j % 4) * 128:(j % 4 + 1) * 128],
                                         wv_sb[:, t, :],
                                         start=(t == 0), stop=False)
                    nc.tensor.matmul(psv[:, 0:130], ones_sb[0:1, 0:128], bv_sb,
                                     start=False, stop=True)
                    if j % 2 == 0:
                        nc.vector.tensor_scalar(VA[:, j, :], psv[:, 0:130],
                                                m01_sb[:, j:j + 1], None,
                                                op0=AluOpType.mult)
                    else:
                        nc.scalar.mul(VA[:, j, :], psv[:, 0:130],
                                      m01_sb[:, j:j + 1])

                # Schraudolph fast-exp in bf16 domain (DVE path, odd j):
                # exp(x) ~= bitcast_bf16(int16(x * 2^7/ln2 + (127*2^7 - C)))
                EXP_A = 184.6650292
                EXP_B = float(127 * (1 << 7)) - 5.5918

                def attn_j(j, qsl, acc0, acc1):
                    ksl = bass.ds(j * 128, 128)
                    pt = pl.tile([128, 2 * CHUNK], FP32, tag="pt")
                    nc.tensor.matmul(pt[:, 0:CHUNK],
                                     KhT[0:64, ksl], QhT[0:64, qsl],
                                     start=True, stop=True,
                                     tile_position=(0, 0))
                    nc.tensor.matmul(pt[:, CHUNK:2 * CHUNK],
                                     KhT[64:128, ksl], QhT[64:128, qsl],
                                     start=True, stop=True,
                                     tile_position=(64, 0))
                    # head0 exp on ScalarE (exact), head1 on DVE (fast-exp),
                    # concurrently on the two halves of the same tile
                    et = sexp.tile([128, 2 * CHUNK], BF16, tag="expT")
                    nc.scalar.activation(et[:, 0:CHUNK], pt[:, 0:CHUNK], AF.Exp)
                    eiv = et.bitcast(I16)
                    nc.vector.tensor_scalar(eiv[:, CHUNK:2 * CHUNK],
                                            pt[:, CHUNK:2 * CHUNK],
                                            EXP_A, EXP_B,
                                            op0=AluOpType.mult,
                                            op1=AluOpType.add)
                    nc.tensor.matmul(acc0, VA[:, j, 0:65], et[:, 0:CHUNK],
                                     start=(j == 0), stop=(j == nskp - 1))
                    nc.tensor.matmul(acc1, VA[:, j, 65:130], et[:, CHUNK:2 * CHUNK],
                                     start=(j == 0), stop=(j == nskp - 1))

                def wo_combine(rti, outw, ch, qt):
                    gq = ch * 4 + qt
                    csl = bass.ds(qt * 128, 128)
                    psb = pw.tile([128, 2 * CHUNK], FP32, tag="ps")
                    pso0 = psb[:, 0:CHUNK]
                    pso1 = psb[:, CHUNK:2 * CHUNK]
                    # 64x128 row-tiled pair, issued adjacently
                    nc.tensor.matmul(pso0, outw[0:64, csl], wo_sb[0:64, :],
                                     start=True, stop=True,
                                     tile_position=(0, 0))
                    nc.tensor.matmul(pso1, outw[64:128, csl], wo_sb[64:128, :],
                                     start=True, stop=True,
                                     tile_position=(64, 0))
                    tmp = sout.tile([128, DM], FP32, tag="tmp")
                    nc.scalar.mul(tmp, pso0, rti[:, qt:qt + 1])
                    ot = sout.tile([128, DM], FP32, tag="ot")
                    nc.vector.scalar_tensor_tensor(
                        ot, pso1, rti[:, 4 + qt:5 + qt], tmp,
                        op0=AluOpType.mult, op1=AluOpType.add)
                    nc.sync.dma_start(out=out[gq * 128:(gq + 1) * 128, :], in_=ot)

                pending = None
                for ch in range(NCH):
                    qsl = bass.ds(ch * CHUNK, CHUNK)
                    def vgroup(g):
                        jj0 = 4 * g
                        njj = kblks(g)
                        for base in range(jj0, jj0 + njj, 2):
                            psb = pw.tile([128, 2 * CHUNK], FP32, tag="ps")
                            proj_v(base, psb[:, 0:CHUNK])
                            if base + 1 < jj0 + njj:
                                proj_v(base + 1, psb[:, CHUNK:2 * CHUNK])

                    if ch == 0:
                        proj_k(0)
                        vgroup(0)
                        proj_qk(QhT, wq_sb, bqk_sb[0:1, 0:128], qT_sb, 0)
                    if ch + 1 < NCH:
                        proj_qk(QhT, wq_sb, bqk_sb[0:1, 0:128], qT_sb, ch + 1)

                    acc0 = pacc.tile([65, CHUNK], FP32, tag="acc0")
                    acc1 = pacc.tile([65, CHUNK], FP32, tag="acc1")
                    for j in range(nskp):
                        if ch == 0 and j % 4 == 0 and j > 0:
                            # feed the rest of the K/V projections just in time
                            g = j // 4
                            proj_k(g)
                            vgroup(g)
                        if pending is not None and j in wo_slot:
                            wo_combine(pending[0], pending[1], pending[2],
                                       wo_slot[j])
                        attn_j(j, qsl, acc0, acc1)

                    # stage acc on ScalarE (fp32 -> early PSUM release): rows
                    # 0:64 = attention out (both heads), row 64 = rowsums
                    stg = srs.tile([65, 2 * CHUNK], FP32, tag="stg")
                    nc.scalar.copy(stg[:, 0:CHUNK], acc0)
                    nc.scalar.copy(stg[:, CHUNK:2 * CHUNK], acc1)
                    # bf16 staging for the Wo lhsT: head0 -> outw[0:64] (DVE
                    # cast), head1 -> cast then DMA partition-shift to 64:128
                    # so the Wo pair row-tiles.
                    outw = sow.tile([128, CHUNK], BF16, tag="outw")
                    otmp = sow.tile([64, CHUNK], BF16, tag="otmp")
                    nc.vector.tensor_copy(outw[0:64, :], stg[0:64, 0:CHUNK])
                    nc.vector.tensor_copy(otmp, stg[0:64, CHUNK:2 * CHUNK])
                    nc.sync.dma_start(out=outw[64:128, :], in_=otmp)
                    # transpose rowsums to partitions: rt[p, h*4+qt] = rs_h[qt*128+p]
                    rt = srs.tile([128, 8], FP32, tag="rt")
                    for h in range(2):
                        for qt in range(4):
                            nc.sync.dma_start(
                                out=rt[:, h * 4 + qt:h * 4 + qt + 1],
                                in_=stg[64:65, h * CHUNK + qt * 128:
                                        h * CHUNK + (qt + 1) * 128])
                    rti = srs.tile([128, 8], FP32, tag="rti")
                    nc.vector.reciprocal(rti, rt)
                    pending = (rti, outw, ch)
                for qt in range(4):
                    wo_combine(pending[0], pending[1], pending[2], qt)
    nc.compile()
    return nc


def _prep_core_inputs(c, q, k, v, keep, nskp, Wq, bq, Wk, bk, Wv, bv, Wo):
    b, hp = divmod(c, 4)
    sl = slice(hp * 128, (hp + 1) * 128)
    bf = ml_dtypes.bfloat16
    scale = 1.0 / np.sqrt(np.float32(D))
    kch = (nskp + 3) // 4
    skp = kch * CHUNK

    def packT(x, nch):
        # [Spad, DM] -> transpose -> [nch, 128, NT, CHUNK] chunk-major contiguous
        xt = x.T.reshape(NT, 128, nch, CHUNK).transpose(2, 1, 0, 3)
        return np.ascontiguousarray(xt).astype(bf)

    idx = keep[b]
    nkeep = idx.shape[0]
    kc = np.zeros((skp, DM), np.float32)
    vc = np.zeros((skp, DM), np.float32)
    kc[:nkeep] = k[b][idx]
    vc[:nkeep] = v[b][idx]
    qTb = packT(q[b], NCH)
    kTb = packT(kc, kch)
    vTb = packT(vc, kch)
    valid = np.zeros((nskp * 128,), np.float32)
    valid[:nkeep] = 1.0
    m01c = np.ascontiguousarray(valid.reshape(nskp, 128).T).astype(np.float32)

    wq_c = np.ascontiguousarray(Wq[:, sl] * scale).astype(bf)
    wk_c = np.ascontiguousarray(Wk[:, sl]).astype(bf)
    wvs = Wv[:, sl]
    wv_c = np.zeros((DM, 130), np.float32)
    wv_c[:, 0:64] = wvs[:, 0:64]
    wv_c[:, 65:129] = wvs[:, 64:128]
    wv_c = wv_c.astype(bf)
    bqk_c = np.concatenate([bq[sl] * scale, bk[sl]]).reshape(1, 256).astype(bf)
    bv_c = np.zeros((1, 130), np.float32)
    bv_c[0, 0:64] = bv[sl][0:64]
    bv_c[0, 64] = 1.0
    bv_c[0, 65:129] = bv[sl][64:128]
    bv_c[0, 129] = 1.0
    bv_c = bv_c.astype(bf)
    wo_c = np.ascontiguousarray(Wo[sl, :]).astype(bf)
    return {"qT": qTb, "kT": kTb, "vT": vTb, "m01": m01c, "wq": wq_c, "wk": wk_c,
            "wv": wv_c, "bqk": bqk_c, "bv": bv_c, "wo": wo_c}


LAST_RESULT = None


def kernel(q, k, v, mask, Wq, bq, Wk, bk, Wv, bv, Wo, bo):
    global LAST_RESULT
    f32 = np.float32
    q, k, v, mask = (np.asarray(x, f32) for x in (q, k, v, mask))
    Wq, bq, Wk, bk, Wv, bv, Wo, bo = (
        np.asarray(x, f32) for x in (Wq, bq, Wk, bk, Wv, bv, Wo, bo))

    # compact keys: masked positions have softmax weight exactly 0
    keep = [np.nonzero(mask[b, 0, 0, :] < 0.5)[0] for b in range(B)]
    nskp = max(9, max((len(ix) + 127) // 128 for ix in keep))

    with_bias = bool(np.any(bq) or np.any(bk) or np.any(bv))
    key = ("nc", with_bias, nskp)
    if key not in _CACHE:
        _CACHE[key] = _build(with_bias, nskp)
    nc = _CACHE[key]

    in_maps = [_prep_core_inputs(c, q, k, v, keep, nskp, Wq, bq, Wk, bk, Wv, bv, Wo)
               for c in range(NCORES)]
    res = bass_utils.run_bass_kernel_spmd(nc, in_maps, core_ids=list(range(NCORES)))
    LAST_RESULT = res
    out = np.zeros((B, S, DM), f32)
    for c in range(NCORES):
        out[c // 4] += np.asarray(res.results[c]["out"], f32)
    out += bo
    return out


# revision 9
# speedup vs baseline: 1.1196x; 1.1196x over previous
"""Multi-head attention (B=2, S=4096, DM=512, H=8) on 8 trn2 NeuronCores.

Sharding: data + head parallel. Core c handles batch b = c//4 and head pair
hp = c%4 (heads 2hp, 2hp+1 = a 128-wide slice of the model dim). Each core
computes its two heads' full attention plus the partial output projection
(its 128 rows of Wo); the host sums the 4 partials per batch and adds bo.

v5:
  * Key compaction: the padding mask is known on the host, and masked keys
    get softmax weight exactly 0 in the reference (logit - 1e9). k/v are
    compacted to the unmasked keys (padded to whole 128-key blocks, zero
    rows beyond): NSKP ~ 29 blocks instead of 32.
  * One [128, 1024] logits tile per j (both heads side by side) from a
    3-deep PSUM pool that also serves the Wo partials and the projection
    scratch (every allocation is the same 2-bank shape, so the Wo/proj
    spikes do not steal logits depth unpredictably). Depth 3 hides the
    ~450ns PE-drain+semaphore latency of the QK->exp handoff that bound
    v2 (depth 2) at ~1130ns/j.
  * Whole-tile exp alternating per j: even j on ScalarE (exact spline Exp),
    odd j on the DVE (one-op Schraudolph int16-bitcast fast-exp). The two
    engines work on consecutive j concurrently; one instruction per tile
    keeps the per-instruction constants (352cyc ScalarE / 120cyc DVE) at
    half the per-head-split cost.
  * QK pair and Wo pair issued adjacently with explicit 64x128 tile
    positions (0,0)/(64,0).
  * ScalarE absorbs the acc->stg copies, the K-proj copy, and the
    per-partition rowsum scale of pso0; the DVE keeps the fast-exp, the
    stt combine, the Q-proj copy and the bf16 casts. V-proj mask scaling
    alternates engines (chunk-0 relief).
"""
import numpy as np
import ml_dtypes

import concourse.bass as bass
from concourse import bacc
import concourse.mybir as mybir
import concourse.tile as tile
from concourse import bass_utils
from concourse.alu_op_type import AluOpType

FP32 = mybir.dt.float32
BF16 = mybir.dt.bfloat16
I16 = mybir.dt.int16
AF = mybir.ActivationFunctionType

B, S, DM, H = 2, 4096, 512, 8
D = DM // H              # 64
NCORES = 8
CHUNK = 512              # q columns processed per attention chunk
NCH = S // CHUNK         # 8
NT = DM // 128           # 4 dm tiles

_CACHE = {}


def _build(with_bias, nskp):
    """nskp = number of 128-key blocks after compaction (<= 32)."""
    kch = (nskp + 3) // 4         # 512-key chunks for K/V (last may be partial)
    nc = bacc.Bacc("TRN2", target_bir_lowering=False, debug=False)

    # chunk-major: [ch, p, t, c] so each chunk's DMA has 4KB-contiguous rows
    qT = nc.dram_tensor("qT", [NCH, 128, NT, CHUNK], BF16, kind="ExternalInput")
    kT = nc.dram_tensor("kT", [kch, 128, NT, CHUNK], BF16, kind="ExternalInput")
    vT = nc.dram_tensor("vT", [kch, 128, NT, CHUNK], BF16, kind="ExternalInput")
    m01 = nc.dram_tensor("m01", [128, nskp], FP32, kind="ExternalInput")
    wq = nc.dram_tensor("wq", [DM, 128], BF16, kind="ExternalInput")
    wk = nc.dram_tensor("wk", [DM, 128], BF16, kind="ExternalInput")
    wv = nc.dram_tensor("wv", [DM, 130], BF16, kind="ExternalInput")
    bqk = nc.dram_tensor("bqk", [1, 256], BF16, kind="ExternalInput")  # bq|bk
    bv = nc.dram_tensor("bv", [1, 130], BF16, kind="ExternalInput")
    wo = nc.dram_tensor("wo", [128, DM], BF16, kind="ExternalInput")
    out = nc.dram_tensor("out", [S, DM], FP32, kind="ExternalOutput")

    def kblks(g):  # blocks in K/V group g
        return min(4, nskp - 4 * g)

    # wo_combine slots inside the next chunk's j loop: 4 evenly spaced js
    gap = max(1, (nskp - 5) // 3)
    wo_slot = {4 + i * gap: i for i in range(4)}
    assert len(wo_slot) == 4 and max(wo_slot) <= nskp - 1

    with tile.TileContext(nc) as tc:
        with tc.tile_pool(name="consts", bufs=1) as consts, \
             tc.tile_pool(name="acts", bufs=1) as acts:
            # ---- first chunk's activations land before anything else ----
            qT_sb = acts.tile([128, NCH, NT, CHUNK], BF16)
            kT_sb = acts.tile([128, kch, NT, CHUNK], BF16)
            vT_sb = acts.tile([128, kch, NT, CHUNK], BF16)
            for p0 in (0, 64):
                nc.sync.dma_start(out=kT_sb[p0:p0 + 64, 0], in_=kT[0][p0:p0 + 64])
                nc.sync.dma_start(out=vT_sb[p0:p0 + 64, 0], in_=vT[0][p0:p0 + 64])
                nc.sync.dma_start(out=qT_sb[p0:p0 + 64, 0], in_=qT[0][p0:p0 + 64])

            # ---- tiny constants; warm the Exp table set during the DMA phase ----
            ones_sb = consts.tile([1, CHUNK], BF16)
            nc.vector.memset(ones_sb, 1.0)
            warm = consts.tile([1, 1], FP32)
            nc.scalar.activation(warm, ones_sb[0:1, 0:1], AF.Exp)

            wq_sb = consts.tile([128, NT, 128], BF16)
            wk_sb = consts.tile([128, NT, 128], BF16)
            wv_sb = consts.tile([128, NT, 130], BF16)
            bqk_sb = consts.tile([1, 256], BF16)
            bv_sb = consts.tile([1, 130], BF16)
            wo_sb = consts.tile([128, DM], BF16)     # rows 0:64 h0, 64:128 h1
            m01_sb = consts.tile([128, nskp], FP32)
            for t in range(NT):
                nc.sync.dma_start(out=wk_sb[:, t, :], in_=wk[t * 128:(t + 1) * 128, :])
                nc.sync.dma_start(out=wv_sb[:, t, :], in_=wv[t * 128:(t + 1) * 128, :])
                nc.sync.dma_start(out=wq_sb[:, t, :], in_=wq[t * 128:(t + 1) * 128, :])
            nc.sync.dma_start(out=bv_sb, in_=bv[:, :])
            nc.sync.dma_start(out=m01_sb, in_=m01[:, :])
            if with_bias:
                nc.sync.dma_start(out=bqk_sb, in_=bqk[:, :])

            # ---- remaining activation DMAs ----
            nc.sync.dma_start(out=wo_sb, in_=wo[:, :])
            for ch in range(1, kch):
                cols = CHUNK if ch < kch - 1 else (nskp - 4 * (kch - 1)) * 128
                nc.sync.dma_start(out=kT_sb[:, ch, :, 0:cols], in_=kT[ch][:, :, 0:cols])
                nc.sync.dma_start(out=vT_sb[:, ch, :, 0:cols], in_=vT[ch][:, :, 0:cols])
            for ch in range(1, NCH):
                nc.sync.dma_start(out=qT_sb[:, ch], in_=qT[ch])

            QhT = acts.tile([128, S], BF16)
            KhT = acts.tile([128, kch * CHUNK], BF16)
            VA = acts.tile([128, nskp, 130], BF16)

            with tc.tile_pool(name="pbig", bufs=3, space="PSUM") as pb, \
                 tc.tile_pool(name="pacc", bufs=1, space="PSUM") as pacc, \
                 tc.tile_pool(name="sexp", bufs=4) as sexp, \
                 tc.tile_pool(name="sout", bufs=3) as sout, \
                 tc.tile_pool(name="sow", bufs=2) as sow, \
                 tc.tile_pool(name="srs", bufs=2) as srs:

                def proj_qk(dst, w_sb, brow, x_sb, ch, cols=CHUNK, on_dve=True):
                    psb = pb.tile([128, 2 * CHUNK], FP32, tag="ps")
                    ps = psb[:, 0:CHUNK]
                    sl = bass.ds(ch * CHUNK, cols)
                    for t in range(NT):
                        nc.tensor.matmul(ps[:, 0:cols], w_sb[:, t, :],
                                         x_sb[:, ch, t, 0:cols],
                                         start=(t == 0),
                                         stop=(t == NT - 1 and not with_bias))
                    if with_bias:
                        nc.tensor.matmul(ps[:, 0:cols], brow, ones_sb[:, 0:cols],
                                         start=False, stop=True)
                    if on_dve:
                        nc.vector.tensor_copy(dst[:, sl], ps[:, 0:cols])
                    else:
                        nc.scalar.copy(dst[:, sl], ps[:, 0:cols])

                def proj_k(g):
                    proj_qk(KhT, wk_sb, bqk_sb[0:1, 128:256], kT_sb, g,
                            kblks(g) * 128, on_dve=False)

                def proj_v(j, psv):
                    # V bias matmul always runs: it also writes the ones columns
                    # (cols 64/129) that produce the attention rowsums.
                    for t in range(NT):
                        nc.tensor.matmul(psv[:, 0:130],
                                         vT_sb[:, j // 4, t,
                                               (j % 4) * 128:(j % 4 + 1) * 128],
                                         wv_sb[:, t, :],
                                         start=(t == 0), stop=False)
                    nc.tensor.matmul(psv[:, 0:130], ones_sb[0:1, 0:128], bv_sb,
                                     start=False, stop=True)
                    if j % 2 == 0:
                        nc.vector.tensor_scalar(VA[:, j, :], psv[:, 0:130],
                                                m01_sb[:, j:j + 1], None,
                                                op0=AluOpType.mult)
                    else:
                        nc.scalar.mul(VA[:, j, :], psv[:, 0:130],
                                      m01_sb[:, j:j + 1])

                # Schraudolph fast-exp in bf16 domain (DVE path, odd j):
                # exp(x) ~= bitcast_bf16(int16(x * 2^7/ln2 + (127*2^7 - C)))
                EXP_A = 184.6650292
                EXP_B = float(127 * (1 << 7)) - 5.5918

                def attn_j(j, qsl, acc0, acc1):
                    ctx = tc.high_priority(offset=2000)
                    ctx.__enter__()
                    ksl = bass.ds(j * 128, 128)
                    pt = pb.tile([128, 2 * CHUNK], FP32, tag="ps")
                    nc.tensor.matmul(pt[:, 0:CHUNK],
                                     KhT[0:64, ksl], QhT[0:64, qsl],
                                     start=True, stop=True,
                                     tile_position=(0, 0))
                    nc.tensor.matmul(pt[:, CHUNK:2 * CHUNK],
                                     KhT[64:128, ksl], QhT[64:128, qsl],
                                     start=True, stop=True,
                                     tile_position=(64, 0))
                    # head0 exp on ScalarE (exact), head1 on DVE (fast-exp),
                    # concurrently on the two halves of the same tile
                    et = sexp.tile([128, 2 * CHUNK], BF16, tag="expT")
                    nc.scalar.activation(et[:, 0:CHUNK], pt[:, 0:CHUNK], AF.Exp)
                    eiv = et.bitcast(I16)
                    nc.vector.tensor_scalar(eiv[:, CHUNK:2 * CHUNK],
                                            pt[:, CHUNK:2 * CHUNK],
                                            EXP_A, EXP_B,
                                            op0=AluOpType.mult,
                                            op1=AluOpType.add)
                    nc.tensor.matmul(acc0, VA[:, j, 0:65], et[:, 0:CHUNK],
                                     start=(j == 0), stop=(j == nskp - 1))
                    nc.tensor.matmul(acc1, VA[:, j, 65:130], et[:, CHUNK:2 * CHUNK],
                                     start=(j == 0), stop=(j == nskp - 1))
                    ctx.__exit__(None, None, None)

                def wo_combine(rti, outw, ch, qt):
                    gq = ch * 4 + qt
                    csl = bass.ds(qt * 128, 128)
                    psb = pb.tile([128, 2 * CHUNK], FP32, tag="ps")
                    pso0 = psb[:, 0:CHUNK]
                    pso1 = psb[:, CHUNK:2 * CHUNK]
                    # 64x128 row-tiled pair, issued adjacently
                    nc.tensor.matmul(pso0, outw[0:64, csl], wo_sb[0:64, :],
                                     start=True, stop=True,
                                     tile_position=(0, 0))
                    nc.tensor.matmul(pso1, outw[64:128, csl], wo_sb[64:128, :],
                                     start=True, stop=True,
                                     tile_position=(64, 0))
                    tmp = sout.tile([128, DM], FP32, tag="tmp")
                    nc.scalar.mul(tmp, pso0, rti[:, qt:qt + 1])
                    ot = sout.tile([128, DM], FP32, tag="ot")
                    nc.vector.scalar_tensor_tensor(
                        ot, pso1, rti[:, 4 + qt:5 + qt], tmp,
                        op0=AluOpType.mult, op1=AluOpType.add)
                    nc.sync.dma_start(out=out[gq * 128:(gq + 1) * 128, :], in_=ot)

                pending = None
                for ch in range(NCH):
                    qsl = bass.ds(ch * CHUNK, CHUNK)
                    def vgroup(g):
                        jj0 = 4 * g
                        njj = kblks(g)
                        for base in range(jj0, jj0 + njj, 2):
                            psb = pb.tile([128, 2 * CHUNK], FP32, tag="ps")
                            proj_v(base, psb[:, 0:CHUNK])
                            if base + 1 < jj0 + njj:
                                proj_v(base + 1, psb[:, CHUNK:2 * CHUNK])

                    if ch == 0:
                        proj_k(0)
                        vgroup(0)
                        proj_qk(QhT, wq_sb, bqk_sb[0:1, 0:128], qT_sb, 0)
                    if ch + 1 < NCH:
                        proj_qk(QhT, wq_sb, bqk_sb[0:1, 0:128], qT_sb, ch + 1)

                    acc0 = pacc.tile([65, CHUNK], FP32, tag="acc0")
                    acc1 = pacc.tile([65, CHUNK], FP32, tag="acc1")
                    for j in range(nskp):
                        if ch == 0 and j % 4 == 0 and j > 0:
                            # feed the rest of the K/V projections just in time
                            g = j // 4
                            proj_k(g)
                            vgroup(g)
                        if pending is not None and j in wo_slot:
                            wo_combine(pending[0], pending[1], pending[2],
                                       wo_slot[j])
                        attn_j(j, qsl, acc0, acc1)

                    # stage acc on ScalarE (fp32 -> early PSUM release): rows
                    # 0:64 = attention out (both heads), row 64 = rowsums
                    stg = srs.tile([65, 2 * CHUNK], FP32, tag="stg")
                    nc.scalar.copy(stg[:, 0:CHUNK], acc0)
                    nc.scalar.copy(stg[:, CHUNK:2 * CHUNK], acc1)
                    # bf16 staging for the Wo lhsT: head0 -> outw[0:64] (DVE
                    # cast), head1 -> cast then DMA partition-shift to 64:128
                    # so the Wo pair row-tiles.
                    outw = sow.tile([128, CHUNK], BF16, tag="outw")
                    otmp = sow.tile([64, CHUNK], BF16, tag="otmp")
                    nc.vector.tensor_copy(outw[0:64, :], stg[0:64, 0:CHUNK])
                    nc.vector.tensor_copy(otmp, stg[0:64, CHUNK:2 * CHUNK])
                    nc.sync.dma_start(out=outw[64:128, :], in_=otmp)
                    # transpose rowsums to partitions: rt[p, h*4+qt] = rs_h[qt*128+p]
                    rt = srs.tile([128, 8], FP32, tag="rt")
                    for h in range(2):
                        for qt in range(4):
                            nc.sync.dma_start(
                                out=rt[:, h * 4 + qt:h * 4 + qt + 1],
                                in_=stg[64:65, h * CHUNK + qt * 128:
                                        h * CHUNK + (qt + 1) * 128])
                    rti = srs.tile([128, 8], FP32, tag="rti")
                    nc.vector.reciprocal(rti, rt)
                    pending = (rti, outw, ch)
                for qt in range(4):
                    wo_combine(pending[0], pending[1], pending[2], qt)
    nc.compile()
    return nc


def _prep_core_inputs(c, q, k, v, keep, nskp, Wq, bq, Wk, bk, Wv, bv, Wo):
    b, hp = divmod(c, 4)
    sl = slice(hp * 128, (hp + 1) * 128)
    bf = ml_dtypes.bfloat16
    scale = 1.0 / np.sqrt(np.float32(D))
    kch = (nskp + 3) // 4
    skp = kch * CHUNK

    def packT(x, nch):
        # [Spad, DM] -> transpose -> [nch, 128, NT, CHUNK] chunk-major contiguous
        xt = x.T.reshape(NT, 128, nch, CHUNK).transpose(2, 1, 0, 3)
        return np.ascontiguousarray(xt).astype(bf)

    idx = keep[b]
    nkeep = idx.shape[0]
    kc = np.zeros((skp, DM), np.float32)
    vc = np.zeros((skp, DM), np.float32)
    kc[:nkeep] = k[b][idx]
    vc[:nkeep] = v[b][idx]
    qTb = packT(q[b], NCH)
    kTb = packT(kc, kch)
    vTb = packT(vc, kch)
    valid = np.zeros((nskp * 128,), np.float32)
    valid[:nkeep] = 1.0
    m01c = np.ascontiguousarray(valid.reshape(nskp, 128).T).astype(np.float32)

    wq_c = np.ascontiguousarray(Wq[:, sl] * scale).astype(bf)
    wk_c = np.ascontiguousarray(Wk[:, sl]).astype(bf)
    wvs = Wv[:, sl]
    wv_c = np.zeros((DM, 130), np.float32)
    wv_c[:, 0:64] = wvs[:, 0:64]
    wv_c[:, 65:129] = wvs[:, 64:128]
    wv_c = wv_c.astype(bf)
    bqk_c = np.concatenate([bq[sl] * scale, bk[sl]]).reshape(1, 256).astype(bf)
    bv_c = np.zeros((1, 130), np.float32)
    bv_c[0, 0:64] = bv[sl][0:64]
    bv_c[0, 64] = 1.0
    bv_c[0, 65:129] = bv[sl][64:128]
    bv_c[0, 129] = 1.0
    bv_c = bv_c.astype(bf)
    wo_c = np.ascontiguousarray(Wo[sl, :]).astype(bf)
    return {"qT": qTb, "kT": kTb, "vT": vTb, "m01": m01c, "wq": wq_c, "wk": wk_c,
            "wv": wv_c, "bqk": bqk_c, "bv": bv_c, "wo": wo_c}


LAST_RESULT = None


def kernel(q, k, v, mask, Wq, bq, Wk, bk, Wv, bv, Wo, bo):
    global LAST_RESULT
    f32 = np.float32
    q, k, v, mask = (np.asarray(x, f32) for x in (q, k, v, mask))
    Wq, bq, Wk, bk, Wv, bv, Wo, bo = (
        np.asarray(x, f32) for x in (Wq, bq, Wk, bk, Wv, bv, Wo, bo))

    # compact keys: masked positions have softmax weight exactly 0
    keep = [np.nonzero(mask[b, 0, 0, :] < 0.5)[0] for b in range(B)]
    nskp = max(9, max((len(ix) + 127) // 128 for ix in keep))

    with_bias = bool(np.any(bq) or np.any(bk) or np.any(bv))
    key = ("nc", with_bias, nskp)
    if key not in _CACHE:
        _CACHE[key] = _build(with_bias, nskp)
    nc = _CACHE[key]

    in_maps = [_prep_core_inputs(c, q, k, v, keep, nskp, Wq, bq, Wk, bk, Wv, bv, Wo)
               for c in range(NCORES)]
    res = bass_utils.run_bass_kernel_spmd(nc, in_maps, core_ids=list(range(NCORES)))
    LAST_RESULT = res
    out = np.zeros((B, S, DM), f32)
    for c in range(NCORES):
        out[c // 4] += np.asarray(res.results[c]["out"], f32)
    out += bo
    return out


# revision 11
# speedup vs baseline: 1.1716x; 1.0465x over previous
"""Multi-head attention (B=2, S=4096, DM=512, H=8) on 8 trn2 NeuronCores.

Sharding: data + head parallel. Core c handles batch b = c//4 and head pair
hp = c%4 (heads 2hp, 2hp+1 = a 128-wide slice of the model dim). Each core
computes its two heads' full attention plus the partial output projection
(its 128 rows of Wo); the host sums the 4 partials per batch and adds bo.

v5:
  * Key compaction: the padding mask is known on the host, and masked keys
    get softmax weight exactly 0 in the reference (logit - 1e9). k/v are
    compacted to the unmasked keys (padded to whole 128-key blocks, zero
    rows beyond): NSKP ~ 29 blocks instead of 32.
  * One [128, 1024] logits tile per j (both heads side by side) from a
    3-deep PSUM pool that also serves the Wo partials and the projection
    scratch (every allocation is the same 2-bank shape, so the Wo/proj
    spikes do not steal logits depth unpredictably). Depth 3 hides the
    ~450ns PE-drain+semaphore latency of the QK->exp handoff that bound
    v2 (depth 2) at ~1130ns/j.
  * Whole-tile exp alternating per j: even j on ScalarE (exact spline Exp),
    odd j on the DVE (one-op Schraudolph int16-bitcast fast-exp). The two
    engines work on consecutive j concurrently; one instruction per tile
    keeps the per-instruction constants (352cyc ScalarE / 120cyc DVE) at
    half the per-head-split cost.
  * QK pair and Wo pair issued adjacently with explicit 64x128 tile
    positions (0,0)/(64,0).
  * ScalarE absorbs the acc->stg copies, the K-proj copy, and the
    per-partition rowsum scale of pso0; the DVE keeps the fast-exp, the
    stt combine, the Q-proj copy and the bf16 casts. V-proj mask scaling
    alternates engines (chunk-0 relief).
"""
import numpy as np
import ml_dtypes

import concourse.bass as bass
from concourse import bacc
import concourse.mybir as mybir
import concourse.tile as tile
from concourse import bass_utils
from concourse.alu_op_type import AluOpType

FP32 = mybir.dt.float32
BF16 = mybir.dt.bfloat16
I16 = mybir.dt.int16
AF = mybir.ActivationFunctionType

B, S, DM, H = 2, 4096, 512, 8
D = DM // H              # 64
NCORES = 8
CHUNK = 512              # q columns processed per attention chunk
NCH = S // CHUNK         # 8
NT = DM // 128           # 4 dm tiles

_CACHE = {}


def _build(with_bias, nskp):
    """nskp = number of 128-key blocks after compaction (<= 32)."""
    kch = (nskp + 3) // 4         # 512-key chunks for K/V (last may be partial)
    nc = bacc.Bacc("TRN2", target_bir_lowering=False, debug=False)

    # chunk-major: [ch, p, t, c] so each chunk's DMA has 4KB-contiguous rows
    qT = nc.dram_tensor("qT", [NCH, 128, NT, CHUNK], BF16, kind="ExternalInput")
    kT = nc.dram_tensor("kT", [kch, 128, NT, CHUNK], BF16, kind="ExternalInput")
    vT = nc.dram_tensor("vT", [kch, 128, NT, CHUNK], BF16, kind="ExternalInput")
    m01 = nc.dram_tensor("m01", [128, nskp], FP32, kind="ExternalInput")
    wq = nc.dram_tensor("wq", [DM, 128], BF16, kind="ExternalInput")
    wk = nc.dram_tensor("wk", [DM, 128], BF16, kind="ExternalInput")
    wv = nc.dram_tensor("wv", [DM, 130], BF16, kind="ExternalInput")
    bqk = nc.dram_tensor("bqk", [1, 256], BF16, kind="ExternalInput")  # bq|bk
    bv = nc.dram_tensor("bv", [1, 130], BF16, kind="ExternalInput")
    wo = nc.dram_tensor("wo", [128, DM], BF16, kind="ExternalInput")
    out = nc.dram_tensor("out", [S, DM], FP32, kind="ExternalOutput")

    def kblks(g):  # blocks in K/V group g
        return min(4, nskp - 4 * g)

    # wo_combine slots inside the next chunk's j loop: 4 evenly spaced js
    gap = max(1, (nskp - 5) // 3)
    wo_slot = {4 + i * gap: i for i in range(4)}
    assert len(wo_slot) == 4 and max(wo_slot) <= nskp - 1

    with tile.TileContext(nc) as tc:
        with tc.tile_pool(name="consts", bufs=1) as consts, \
             tc.tile_pool(name="acts", bufs=1) as acts:
            # ---- first chunk's activations land before anything else ----
            qT_sb = acts.tile([128, NCH, NT, CHUNK], BF16)
            kT_sb = acts.tile([128, kch, NT, CHUNK], BF16)
            vT_sb = acts.tile([128, kch, NT, CHUNK], BF16)
            for p0 in (0, 64):
                nc.sync.dma_start(out=kT_sb[p0:p0 + 64, 0], in_=kT[0][p0:p0 + 64])
                nc.sync.dma_start(out=vT_sb[p0:p0 + 64, 0], in_=vT[0][p0:p0 + 64])
                nc.sync.dma_start(out=qT_sb[p0:p0 + 64, 0], in_=qT[0][p0:p0 + 64])

            # ---- tiny constants; warm the Exp table set during the DMA phase ----
            ones_sb = consts.tile([1, CHUNK], BF16)
            nc.vector.memset(ones_sb, 1.0)
            warm = consts.tile([1, 1], FP32)
            nc.scalar.activation(warm, ones_sb[0:1, 0:1], AF.Exp)

            wq_sb = consts.tile([128, NT, 128], BF16)
            wk_sb = consts.tile([128, NT, 128], BF16)
            wv_sb = consts.tile([128, NT, 130], BF16)
            bqk_sb = consts.tile([1, 256], BF16)
            bv_sb = consts.tile([1, 130], BF16)
            wo_sb = consts.tile([128, DM], BF16)     # rows 0:64 h0, 64:128 h1
            m01_sb = consts.tile([128, nskp], FP32)
            for t in range(NT):
                nc.sync.dma_start(out=wk_sb[:, t, :], in_=wk[t * 128:(t + 1) * 128, :])
                nc.sync.dma_start(out=wv_sb[:, t, :], in_=wv[t * 128:(t + 1) * 128, :])
                nc.sync.dma_start(out=wq_sb[:, t, :], in_=wq[t * 128:(t + 1) * 128, :])
            nc.sync.dma_start(out=bv_sb, in_=bv[:, :])
            nc.sync.dma_start(out=m01_sb, in_=m01[:, :])
            if with_bias:
                nc.sync.dma_start(out=bqk_sb, in_=bqk[:, :])

            # ---- remaining activation DMAs ----
            # kv chunks split 4-way across partitions (parallel HW queues),
            # q chunks interleaved roughly by first-need time
            nc.sync.dma_start(out=wo_sb, in_=wo[:, :])
            nc.sync.dma_start(out=qT_sb[:, 1], in_=qT[1])
            q_next = 2
            for ch in range(1, kch):
                cols = CHUNK if ch < kch - 1 else (nskp - 4 * (kch - 1)) * 128
                for p0 in (0, 32, 64, 96):
                    nc.sync.dma_start(out=kT_sb[p0:p0 + 32, ch, :, 0:cols],
                                      in_=kT[ch][p0:p0 + 32, :, 0:cols])
                    nc.sync.dma_start(out=vT_sb[p0:p0 + 32, ch, :, 0:cols],
                                      in_=vT[ch][p0:p0 + 32, :, 0:cols])
                if ch % 2 == 0 and q_next < NCH:
                    nc.sync.dma_start(out=qT_sb[:, q_next], in_=qT[q_next])
                    q_next += 1
            for ch in range(q_next, NCH):
                nc.sync.dma_start(out=qT_sb[:, ch], in_=qT[ch])

            QhT = acts.tile([128, S], BF16)
            KhT = acts.tile([128, kch * CHUNK], BF16)
            VA = acts.tile([128, nskp, 130], BF16)

            with tc.tile_pool(name="pbig", bufs=3, space="PSUM") as pb, \
                 tc.tile_pool(name="pacc", bufs=1, space="PSUM") as pacc, \
                 tc.tile_pool(name="sexp", bufs=4) as sexp, \
                 tc.tile_pool(name="sout", bufs=3) as sout, \
                 tc.tile_pool(name="sow", bufs=2) as sow, \
                 tc.tile_pool(name="srs", bufs=2) as srs:

                def proj_qk(dst, w_sb, brow, x_sb, ch, cols=CHUNK, on_dve=True):
                    psb = pb.tile([128, 2 * CHUNK], FP32, tag="ps")
                    ps = psb[:, 0:CHUNK]
                    sl = bass.ds(ch * CHUNK, cols)
                    for t in range(NT):
                        nc.tensor.matmul(ps[:, 0:cols], w_sb[:, t, :],
                                         x_sb[:, ch, t, 0:cols],
                                         start=(t == 0),
                                         stop=(t == NT - 1 and not with_bias))
                    if with_bias:
                        nc.tensor.matmul(ps[:, 0:cols], brow, ones_sb[:, 0:cols],
                                         start=False, stop=True)
                    if on_dve:
                        nc.vector.tensor_copy(dst[:, sl], ps[:, 0:cols])
                    else:
                        nc.scalar.copy(dst[:, sl], ps[:, 0:cols])

                def proj_k(g):
                    proj_qk(KhT, wk_sb, bqk_sb[0:1, 128:256], kT_sb, g,
                            kblks(g) * 128, on_dve=False)

                def proj_v(j, psv):
                    # V bias matmul always runs: it also writes the ones columns
                    # (cols 64/129) that produce the attention rowsums.
                    for t in range(NT):
                        nc.tensor.matmul(psv[:, 0:130],
                                         vT_sb[:, j // 4, t,
                                               (j % 4) * 128:(j % 4 + 1) * 128],
                                         wv_sb[:, t, :],
                                         start=(t == 0), stop=False)
                    nc.tensor.matmul(psv[:, 0:130], ones_sb[0:1, 0:128], bv_sb,
                                     start=False, stop=True)
                    if j % 2 == 0:
                        nc.vector.tensor_scalar(VA[:, j, :], psv[:, 0:130],
                                                m01_sb[:, j:j + 1], None,
                                                op0=AluOpType.mult)
                    else:
                        nc.scalar.mul(VA[:, j, :], psv[:, 0:130],
                                      m01_sb[:, j:j + 1])

                # Schraudolph fast-exp in bf16 domain (DVE path, odd j):
                # exp(x) ~= bitcast_bf16(int16(x * 2^7/ln2 + (127*2^7 - C)))
                EXP_A = 184.6650292
                EXP_B = float(127 * (1 << 7)) - 5.5918

                def attn_j(j, qsl, acc0, acc1):
                    ctx = tc.high_priority(offset=2000)
                    ctx.__enter__()
                    ksl = bass.ds(j * 128, 128)
                    pt = pb.tile([128, 2 * CHUNK], FP32, tag="ps")
                    nc.tensor.matmul(pt[:, 0:CHUNK],
                                     KhT[0:64, ksl], QhT[0:64, qsl],
                                     start=True, stop=True,
                                     tile_position=(0, 0))
                    nc.tensor.matmul(pt[:, CHUNK:2 * CHUNK],
                                     KhT[64:128, ksl], QhT[64:128, qsl],
                                     start=True, stop=True,
                                     tile_position=(64, 0))
                    # head0 exp on ScalarE (exact), head1 on DVE (fast-exp),
                    # concurrently on the two halves of the same tile
                    et = sexp.tile([128, 2 * CHUNK], BF16, tag="expT")
                    nc.scalar.activation(et[:, 0:CHUNK], pt[:, 0:CHUNK], AF.Exp)
                    eiv = et.bitcast(I16)
                    nc.vector.tensor_scalar(eiv[:, CHUNK:2 * CHUNK],
                                            pt[:, CHUNK:2 * CHUNK],
                                            EXP_A, EXP_B,
                                            op0=AluOpType.mult,
                                            op1=AluOpType.add)
                    nc.tensor.matmul(acc0, VA[:, j, 0:65], et[:, 0:CHUNK],
                                     start=(j == 0), stop=(j == nskp - 1))
                    nc.tensor.matmul(acc1, VA[:, j, 65:130], et[:, CHUNK:2 * CHUNK],
                                     start=(j == 0), stop=(j == nskp - 1))
                    ctx.__exit__(None, None, None)

                def wo_combine(rti, outw, ch, qt):
                    gq = ch * 4 + qt
                    csl = bass.ds(qt * 128, 128)
                    psb = pb.tile([128, 2 * CHUNK], FP32, tag="ps")
                    pso0 = psb[:, 0:CHUNK]
                    pso1 = psb[:, CHUNK:2 * CHUNK]
                    # 64x128 row-tiled pair, issued adjacently
                    nc.tensor.matmul(pso0, outw[0:64, csl], wo_sb[0:64, :],
                                     start=True, stop=True,
                                     tile_position=(0, 0))
                    nc.tensor.matmul(pso1, outw[64:128, csl], wo_sb[64:128, :],
                                     start=True, stop=True,
                                     tile_position=(64, 0))
                    tmp = sout.tile([128, DM], FP32, tag="tmp")
                    nc.scalar.mul(tmp, pso0, rti[:, qt:qt + 1])
                    ot = sout.tile([128, DM], FP32, tag="ot")
                    nc.vector.scalar_tensor_tensor(
                        ot, pso1, rti[:, 4 + qt:5 + qt], tmp,
                        op0=AluOpType.mult, op1=AluOpType.add)
                    nc.sync.dma_start(out=out[gq * 128:(gq + 1) * 128, :], in_=ot)

                pending = None
                for ch in range(NCH):
                    qsl = bass.ds(ch * CHUNK, CHUNK)
                    def vgroup(g):
                        jj0 = 4 * g
                        njj = kblks(g)
                        for base in range(jj0, jj0 + njj, 2):
                            psb = pb.tile([128, 2 * CHUNK], FP32, tag="ps")
                            proj_v(base, psb[:, 0:CHUNK])
                            if base + 1 < jj0 + njj:
                                proj_v(base + 1, psb[:, CHUNK:2 * CHUNK])

                    if ch == 0:
                        proj_k(0)
                        vgroup(0)
                        proj_qk(QhT, wq_sb, bqk_sb[0:1, 0:128], qT_sb, 0)
                    if ch + 1 < NCH:
                        proj_qk(QhT, wq_sb, bqk_sb[0:1, 0:128], qT_sb, ch + 1)

                    acc0 = pacc.tile([65, CHUNK], FP32, tag="acc0")
                    acc1 = pacc.tile([65, CHUNK], FP32, tag="acc1")
                    for j in range(nskp):
                        if ch == 0 and j % 4 == 0 and j > 0:
                            # feed the rest of the K/V projections just in time
                            g = j // 4
                            proj_k(g)
                            vgroup(g)
                        if pending is not None and j in wo_slot:
                            wo_combine(pending[0], pending[1], pending[2],
                                       wo_slot[j])
                        attn_j(j, qsl, acc0, acc1)

                    # stage acc on ScalarE (fp32 -> early PSUM release): rows
                    # 0:64 = attention out (both heads), row 64 = rowsums
                    stg = srs.tile([65, 2 * CHUNK], FP32, tag="stg")
                    nc.scalar.copy(stg[:, 0:CHUNK], acc0)
                    nc.scalar.copy(stg[:, CHUNK:2 * CHUNK], acc1)
                    # bf16 staging for the Wo lhsT: head0 -> outw[0:64] (DVE
                    # cast), head1 -> cast then DMA partition-shift to 64:128
                    # so the Wo pair row-tiles.
                    outw = sow.tile([128, CHUNK], BF16, tag="outw")
                    otmp = sow.tile([64, CHUNK], BF16, tag="otmp")
                    nc.vector.tensor_copy(outw[0:64, :], stg[0:64, 0:CHUNK])
                    nc.vector.tensor_copy(otmp, stg[0:64, CHUNK:2 * CHUNK])
                    nc.sync.dma_start(out=outw[64:128, :], in_=otmp)
                    # transpose rowsums to partitions: rt[p, h*4+qt] = rs_h[qt*128+p]
                    # (split across both hwdge dispatch queues)
                    rt = srs.tile([128, 8], FP32, tag="rt")
                    for h in range(2):
                        eng = nc.sync if h == 0 else nc.scalar
                        for qt in range(4):
                            eng.dma_start(
                                out=rt[:, h * 4 + qt:h * 4 + qt + 1],
                                in_=stg[64:65, h * CHUNK + qt * 128:
                                        h * CHUNK + (qt + 1) * 128])
                    rti = srs.tile([128, 8], FP32, tag="rti")
                    nc.vector.reciprocal(rti, rt)
                    pending = (rti, outw, ch)
                for qt in range(4):
                    wo_combine(pending[0], pending[1], pending[2], qt)
    nc.compile()
    return nc


def _prep_core_inputs(c, q, k, v, keep, nskp, Wq, bq, Wk, bk, Wv, bv, Wo):
    b, hp = divmod(c, 4)
    sl = slice(hp * 128, (hp + 1) * 128)
    bf = ml_dtypes.bfloat16
    scale = 1.0 / np.sqrt(np.float32(D))
    kch = (nskp + 3) // 4
    skp = kch * CHUNK

    def packT(x, nch):
        # [Spad, DM] -> transpose -> [nch, 128, NT, CHUNK] chunk-major contiguous
        xt = x.T.reshape(NT, 128, nch, CHUNK).transpose(2, 1, 0, 3)
        return np.ascontiguousarray(xt).astype(bf)

    idx = keep[b]
    nkeep = idx.shape[0]
    kc = np.zeros((skp, DM), np.float32)
    vc = np.zeros((skp, DM), np.float32)
    kc[:nkeep] = k[b][idx]
    vc[:nkeep] = v[b][idx]
    qTb = packT(q[b], NCH)
    kTb = packT(kc, kch)
    vTb = packT(vc, kch)
    valid = np.zeros((nskp * 128,), np.float32)
    valid[:nkeep] = 1.0
    m01c = np.ascontiguousarray(valid.reshape(nskp, 128).T).astype(np.float32)

    wq_c = np.ascontiguousarray(Wq[:, sl] * scale).astype(bf)
    wk_c = np.ascontiguousarray(Wk[:, sl]).astype(bf)
    wvs = Wv[:, sl]
    wv_c = np.zeros((DM, 130), np.float32)
    wv_c[:, 0:64] = wvs[:, 0:64]
    wv_c[:, 65:129] = wvs[:, 64:128]
    wv_c = wv_c.astype(bf)
    bqk_c = np.concatenate([bq[sl] * scale, bk[sl]]).reshape(1, 256).astype(bf)
    bv_c = np.zeros((1, 130), np.float32)
    bv_c[0, 0:64] = bv[sl][0:64]
    bv_c[0, 64] = 1.0
    bv_c[0, 65:129] = bv[sl][64:128]
    bv_c[0, 129] = 1.0
    bv_c = bv_c.astype(bf)
    wo_c = np.ascontiguousarray(Wo[sl, :]).astype(bf)
    return {"qT": qTb, "kT": kTb, "vT": vTb, "m01": m01c, "wq": wq_c, "wk": wk_c,
            "wv": wv_c, "bqk": bqk_c, "bv": bv_c, "wo": wo_c}


LAST_RESULT = None


def kernel(q, k, v, mask, Wq, bq, Wk, bk, Wv, bv, Wo, bo):
    global LAST_RESULT
    f32 = np.float32
    q, k, v, mask = (np.asarray(x, f32) for x in (q, k, v, mask))
    Wq, bq, Wk, bk, Wv, bv, Wo, bo = (
        np.asarray(x, f32) for x in (Wq, bq, Wk, bk, Wv, bv, Wo, bo))

    # compact keys: masked positions have softmax weight exactly 0
    keep = [np.nonzero(mask[b, 0, 0, :] < 0.5)[0] for b in range(B)]
    nskp = max(9, max((len(ix) + 127) // 128 for ix in keep))

    with_bias = bool(np.any(bq) or np.any(bk) or np.any(bv))
    key = ("nc", with_bias, nskp)
    if key not in _CACHE:
        _CACHE[key] = _build(with_bias, nskp)
    nc = _CACHE[key]

    in_maps = [_prep_core_inputs(c, q, k, v, keep, nskp, Wq, bq, Wk, bk, Wv, bv, Wo)
               for c in range(NCORES)]
    res = bass_utils.run_bass_kernel_spmd(nc, in_maps, core_ids=list(range(NCORES)))
    LAST_RESULT = res
    out = np.zeros((B, S, DM), f32)
    for c in range(NCORES):
        out[c // 4] += np.asarray(res.results[c]["out"], f32)
    out += bo
    return out
